# revision 4
# baseline (speedup 1.0000x reference)
"""DIMKT Bass/Tile kernel for TRN2, 8-core data-parallel over batch.

v2 design (see layout notes):
- Per core B_c=64, S=200, D=512. Activations transposed: [128 part = f%128,
  4 = f//128 blocks, 64 cols = batch].
- Scan computes d = x_t - h on DVE, then s-PSUM = d @ [Ws1 | 2*Ws2] with the
  per-feature biases entering PSUM via tiny K=2 matmuls against the corr
  one-hot (both table rows = bias). One sigmoid ACT over [128,8,64] covers
  sigma(a) and tanh(b) = 2*sigma(2b)-1 (the *2 is folded into weights).
- gamma path: ki PSUM = h @ Wki1; rest of ki_in ([corr_emb, qde, cde] parts
  + b_ki) is a pure table-lookup stream kib produced per chunk by one-hot
  matmuls (E_qd/E_cd/E_corr tables transformed on device at startup).
- x stream per chunk: qe/ce dma_gather + dense matmuls + qd/cd one-hot.
- Scan weights optionally fp8-e4m3 (scaled by 2^k, descale via ACT scale=).
"""
import sys
for p in ('/opt/trn_rl_repo', '/root/.axon_site/_ro/trn_rl_repo'):
    if p not in sys.path:
        sys.path.insert(0, p)

import numpy as np
import ml_dtypes

import concourse.bass as bass
import concourse.mybir as mybir
import concourse.tile as tile
from concourse import bacc
from concourse import bass_utils

BF = mybir.dt.bfloat16
F32 = mybir.dt.float32
F8 = mybir.dt.float8e4
I16 = mybir.dt.int16
I32 = mybir.dt.int32
AF = mybir.ActivationFunctionType
OP = mybir.AluOpType
bf16 = ml_dtypes.bfloat16
f8e4 = ml_dtypes.float8_e4m3fn

FP8_SCAN = False          # scan weights in fp8-e4m3 (scaled)

B, S, D = 512, 200, 512
NCORE = 8
BC = B // NCORE          # 64 batch rows per core
TC = 8                   # timesteps per chunk
CHUNK = TC * BC          # 512 cols per chunk
NCHUNK = S // TC         # 25 chunks
NQ, NC_, NQD, NCD = 10000, 500, 101, 101
NSTEP = S - 1            # 199 scan steps
NSC = (NSTEP + TC - 1) // TC   # 25 scan chunks (last partial)

WDT = F8 if FP8_SCAN else BF
wnp = f8e4 if FP8_SCAN else bf16


def _wtile(w):
    """[K, M] -> [128, K//128, M//128, 128] (lhsT blocks)."""
    K, M = w.shape
    return np.ascontiguousarray(
        w.reshape(K // 128, 128, M // 128, 128).transpose(1, 0, 2, 3))


def _ttile(e):
    """[R, Dm] -> [128, Dm//128, R]  (E^T blocks as lhsT)."""
    R, Dm = e.shape
    return np.ascontiguousarray(
        e.reshape(R, Dm // 128, 128).transpose(2, 1, 0)).astype(bf16)


def _wrap_idx(flat):
    """[NCHUNK*CHUNK] int -> [128, NCHUNK*CHUNK//16] int16 wrapped+replicated."""
    blocks = flat.reshape(NCHUNK, CHUNK // 16, 16)           # [c, j, p]
    w = blocks.transpose(0, 2, 1).reshape(NCHUNK, 16, CHUNK // 16)
    w = np.concatenate(list(w), axis=1)                       # [16, total/16]
    return np.ascontiguousarray(np.tile(w, (8, 1))).astype(np.int16)


def _scale_for(w):
    if not FP8_SCAN:
        return 1.0
    m = float(np.abs(w).max())
    return float(2.0 ** np.floor(np.log2(224.0 / m)))


def build_program():
    nc = bacc.Bacc("TRN2", target_bir_lowering=False, debug=False,
                   num_devices=NCORE)
    dt = nc.dram_tensor
    EQ = dt("EQ", [NQ, D], BF, kind="ExternalInput")
    EC = dt("EC", [NC_, D], BF, kind="ExternalInput")
    QIDX = dt("QIDX", [128, NCHUNK * CHUNK // 16], I16, kind="ExternalInput")
    CIDX = dt("CIDX", [128, NCHUNK * CHUNK // 16], I16, kind="ExternalInput")
    QDI = dt("QDI", [128, NCHUNK * CHUNK], BF, kind="ExternalInput")
    CDI = dt("CDI", [128, NCHUNK * CHUNK], BF, kind="ExternalInput")
    COI = dt("COI", [2, NCHUNK * CHUNK], BF, kind="ExternalInput")
    WS12 = dt("WS12", [128, 4, 8, 128], WDT, kind="ExternalInput")
    WP12 = dt("WP12", [128, 4, 8, 128], WDT, kind="ExternalInput")
    WKI = dt("WKI", [128, 4, 4, 128], WDT, kind="ExternalInput")
    WXQ = dt("WXQ", [128, 4, 4, 128], BF, kind="ExternalInput")
    WXC = dt("WXC", [128, 4, 4, 128], BF, kind="ExternalInput")
    EQDT = dt("EQDT", [128, 4, NQD], BF, kind="ExternalInput")
    WQDR = dt("WQDR", [128, 4, 1024], BF, kind="ExternalInput")
    BQD = dt("BQD", [1, 1024], BF, kind="ExternalInput")
    ECDT = dt("ECDT", [128, 4, NCD], BF, kind="ExternalInput")
    WCDR = dt("WCDR", [128, 4, 1024], BF, kind="ExternalInput")
    ECOT = dt("ECOT", [128, 4, 2], BF, kind="ExternalInput")
    WCOR = dt("WCOR", [128, 4, 1024], BF, kind="ExternalInput")
    BCO = dt("BCO", [1, 1024], BF, kind="ExternalInput")
    WCOK = dt("WCOK", [128, 4, 512], BF, kind="ExternalInput")
    BCOK = dt("BCOK", [1, 512], BF, kind="ExternalInput")
    TBS = dt("TBS", [2, 8, 128], BF, kind="ExternalInput")
    SCL = dt("SCL", [1, 4], F32, kind="ExternalInput")  # unused placeholder
    H0T = dt("H0T", [128, 4, 64], BF, kind="ExternalInput")
    YT = dt("YT", [S, BC], F32, kind="ExternalOutput")

    # scales are compile-time floats, stored on nc for prep symmetry
    inv_s = nc._dimkt_inv_scales  # set by caller before build

    with tile.TileContext(nc) as tc:
        with (
            tc.tile_pool(name="const", bufs=1) as cp,
            tc.tile_pool(name="gather", bufs=2) as gp,
            tc.tile_pool(name="oh", bufs=2) as ohp,
            tc.tile_pool(name="oh3", bufs=3) as oh3,
            tc.tile_pool(name="strm", bufs=3) as stp,
            tc.tile_pool(name="scan", bufs=2) as scp,
            tc.tile_pool(name="sps", bufs=2, space="PSUM") as sps,
            tc.tile_pool(name="pps", bufs=2, space="PSUM") as pps,
            tc.tile_pool(name="gps", bufs=1, space="PSUM") as gps,
            tc.tile_pool(name="bps", bufs=1, space="PSUM") as bps,
            tc.tile_pool(name="yps", bufs=1, space="PSUM") as yps,
        ):
            # ---------------- constants ----------------
            ws12 = cp.tile([128, 4, 8, 128], WDT)
            nc.sync.dma_start(ws12[:], WS12.ap())
            wp12 = cp.tile([128, 4, 8, 128], WDT)
            nc.sync.dma_start(wp12[:], WP12.ap())
            wki = cp.tile([128, 4, 4, 128], WDT)
            nc.sync.dma_start(wki[:], WKI.ap())
            wxq = cp.tile([128, 4, 4, 128], BF)
            nc.sync.dma_start(wxq[:], WXQ.ap())
            wxc = cp.tile([128, 4, 4, 128], BF)
            nc.sync.dma_start(wxc[:], WXC.ap())
            tbs = cp.tile([2, 8, 128], BF)
            nc.sync.dma_start(tbs[:], TBS.ap())
            qidx = cp.tile([128, NCHUNK * CHUNK // 16], I16)
            nc.sync.dma_start(qidx[:], QIDX.ap())
            cidx = cp.tile([128, NCHUNK * CHUNK // 16], I16)
            nc.sync.dma_start(cidx[:], CIDX.ap())
            ones = cp.tile([128, 1], F32)
            nc.gpsimd.memset(ones[:], 1.0)
            ones1 = cp.tile([1, 128], BF)
            nc.gpsimd.memset(ones1[:], 1.0)
            iota_i = cp.tile([128, 1], I32)
            nc.gpsimd.iota(iota_i[:], [[0, 1]], base=0, channel_multiplier=1)
            iota_b = cp.tile([128, 1], BF)
            nc.vector.tensor_copy(iota_b[:], iota_i[:])
            zrow = cp.tile([1, BC], F32)
            nc.gpsimd.memset(zrow[:], 0.0)

            # ---------------- table transforms ----------------
            eqdt = gp.tile([128, 4, NQD], BF, tag="qe")
            nc.sync.dma_start(eqdt[:], EQDT.ap())
            wqdr = gp.tile([128, 4, 1024], BF, tag="ce")
            nc.sync.dma_start(wqdr[:], WQDR.ap())
            bqd = cp.tile([1, 1024], BF)
            nc.sync.dma_start(bqd[:], BQD.ap())
            ecdt = ohp.tile([128, 4, NCD], BF, tag="qdi")
            nc.sync.dma_start(ecdt[:], ECDT.ap())
            wcdr = ohp.tile([128, 4, 1024], BF, tag="cdi")
            nc.sync.dma_start(wcdr[:], WCDR.ap())
            ecot = cp.tile([128, 4, 2], BF)
            nc.sync.dma_start(ecot[:], ECOT.ap())
            wcor = ohp.tile([128, 4, 1024], BF, tag="ohqd")
            nc.sync.dma_start(wcor[:], WCOR.ap())
            bco = cp.tile([1, 1024], BF)
            nc.sync.dma_start(bco[:], BCO.ap())
            wcok = ohp.tile([128, 4, 512], BF, tag="ohcd")
            nc.sync.dma_start(wcok[:], WCOK.ap())
            bcok = cp.tile([1, 512], BF)
            nc.sync.dma_start(bcok[:], BCOK.ap())

            tqd = cp.tile([128, 8, 128], BF)   # rows 0..100 valid
            nc.gpsimd.memset(tqd[:], 0.0)
            tcd = cp.tile([128, 8, 128], BF)
            nc.gpsimd.memset(tcd[:], 0.0)
            tco = cp.tile([2, 8, 128], BF)
            tcok = cp.tile([2, 4, 128], BF)

            for half in range(2):
                hs = slice(half * 512, (half + 1) * 512)
                ms = slice(half * 4, (half + 1) * 4)
                ps = bps.tile([128, 512], F32, tag="prod")
                for k in range(4):
                    nc.tensor.matmul(ps[:NQD], eqdt[:, k, :], wqdr[:, k, hs],
                                     start=(k == 0), stop=False)
                nc.tensor.matmul(ps[:NQD], ones1[:1, :NQD], bqd[:, hs],
                                 start=False, stop=True)
                nc.vector.tensor_copy(
                    tqd[:NQD, ms, :],
                    ps[:NQD].rearrange("p (m j) -> p m j", j=128))
                ps = bps.tile([128, 512], F32, tag="prod")
                for k in range(4):
                    nc.tensor.matmul(ps[:NCD], ecdt[:, k, :], wcdr[:, k, hs],
                                     start=(k == 0), stop=(k == 3))
                nc.vector.tensor_copy(
                    tcd[:NCD, ms, :],
                    ps[:NCD].rearrange("p (m j) -> p m j", j=128))
                ps = bps.tile([128, 512], F32, tag="prod")
                for k in range(4):
                    nc.tensor.matmul(ps[:2], ecot[:, k, :], wcor[:, k, hs],
                                     start=(k == 0), stop=False)
                nc.tensor.matmul(ps[:2], ones1[:1, :2], bco[:, hs],
                                 start=False, stop=True)
                nc.vector.tensor_copy(
                    tco[:, ms, :],
                    ps[:2].rearrange("p (m j) -> p m j", j=128))
            ps = bps.tile([128, 512], F32, tag="prod")
            for k in range(4):
                nc.tensor.matmul(ps[:2], ecot[:, k, :], wcok[:, k, :],
                                 start=(k == 0), stop=False)
            nc.tensor.matmul(ps[:2], ones1[:1, :2], bcok[:],
                             start=False, stop=True)
            nc.vector.tensor_copy(tcok[:],
                                  ps[:2].rearrange("p (m j) -> p m j", j=128))

            # ---------------- chunk producer ----------------
            def produce(c):
                lo = c * CHUNK
                qe = gp.tile([128, 4, CHUNK], BF, tag="qe")
                nc.gpsimd.dma_gather(
                    qe[:], EQ.ap(),
                    qidx[:, c * (CHUNK // 16):(c + 1) * (CHUNK // 16)],
                    CHUNK, CHUNK, D, transpose=True, single_packet=False)
                ce = gp.tile([128, 4, CHUNK], BF, tag="ce")
                nc.gpsimd.dma_gather(
                    ce[:], EC.ap(),
                    cidx[:, c * (CHUNK // 16):(c + 1) * (CHUNK // 16)],
                    CHUNK, CHUNK, D, transpose=True, single_packet=False)
                qdi = ohp.tile([128, CHUNK], BF, tag="qdi")
                nc.sync.dma_start(qdi[:], QDI.ap()[:, lo:lo + CHUNK])
                cdi = ohp.tile([128, CHUNK], BF, tag="cdi")
                nc.sync.dma_start(cdi[:], CDI.ap()[:, lo:lo + CHUNK])
                coi = ohp.tile([2, CHUNK], BF, tag="coi")
                nc.sync.dma_start(coi[:], COI.ap()[:, lo:lo + CHUNK])
                oh_qd = ohp.tile([128, CHUNK], BF, tag="ohqd")
                nc.vector.tensor_tensor(
                    oh_qd[:], iota_b[:, 0:1].to_broadcast((128, CHUNK)), qdi[:],
                    OP.is_equal)
                oh_cd = ohp.tile([128, CHUNK], BF, tag="ohcd")
                nc.vector.tensor_tensor(
                    oh_cd[:], iota_b[:, 0:1].to_broadcast((128, CHUNK)), cdi[:],
                    OP.is_equal)
                oh_co = oh3.tile([2, CHUNK], BF, tag="ohco")
                nc.vector.tensor_tensor(
                    oh_co[:], iota_b[:2, 0:1].to_broadcast((2, CHUNK)), coi[:],
                    OP.is_equal)

                xb = stp.tile([128, 4, CHUNK], BF, tag="xb")
                kib = stp.tile([128, 4, CHUNK], BF, tag="kib")
                for half in range(2):
                    sl = slice(half * 256, (half + 1) * 256)
                    xp = bps.tile([128, 4, 256], F32, tag="prod")
                    for m in range(4):
                        for k in range(4):
                            nc.tensor.matmul(xp[:, m], wxq[:, k, m], qe[:, k, sl],
                                             start=(k == 0), stop=False)
                        for k in range(4):
                            nc.tensor.matmul(xp[:, m], wxc[:, k, m], ce[:, k, sl],
                                             start=False, stop=False)
                        nc.tensor.matmul(xp[:, m], tqd[:, m, :], oh_qd[:, sl],
                                         start=False, stop=False)
                        nc.tensor.matmul(xp[:, m], tcd[:, m, :], oh_cd[:, sl],
                                         start=False, stop=True)
                    nc.scalar.copy(xb[:, :, sl], xp[:])
                    kp = bps.tile([128, 4, 256], F32, tag="prod")
                    for m in range(4):
                        nc.tensor.matmul(kp[:, m], tqd[:, 4 + m, :], oh_qd[:, sl],
                                         start=True, stop=False)
                        nc.tensor.matmul(kp[:, m], tcd[:, 4 + m, :], oh_cd[:, sl],
                                         start=False, stop=False)
                        nc.tensor.matmul(kp[:, m], tcok[:, m, :], oh_co[:, sl],
                                         start=False, stop=True)
                    nc.vector.tensor_copy(kib[:, :, sl], kp[:])
                return xb, kib, oh_co

            chunks = {0: produce(0), 1: produce(1)}

            # ---------------- scan ----------------
            h = scp.tile([128, 4, 64], BF, tag="h")
            nc.sync.dma_start(h[:], H0T.ap())

            for c in range(NSC):
                xb, kib, oh_co = chunks[c]
                steps = min(TC, NSTEP - c * TC)
                prch = scp.tile([128, TC * 64], F32, tag="prch")
                if steps < TC:
                    nc.gpsimd.memset(prch[:], 0.0)
                for tt in range(steps):
                    col = slice(tt * 64, (tt + 1) * 64)
                    d = scp.tile([128, 4, 64], BF, tag="d")
                    nc.vector.tensor_tensor(d[:], xb[:, :, col], h[:],
                                            OP.subtract)
                    sp = sps.tile([128, 8, 64], F32, tag="s")
                    for m in range(8):
                        nc.tensor.matmul(sp[:, m], tbs[:2, m, :], oh_co[:, col],
                                         start=True, stop=False)
                        for k in range(4):
                            nc.tensor.matmul(sp[:, m], ws12[:, k, m], d[:, k],
                                             start=False, stop=(k == 3))
                    u = scp.tile([128, 8, 64], BF, tag="u")
                    nc.scalar.activation(u[:], sp[:], AF.Sigmoid,
                                         scale=inv_s[0])
                    v = scp.tile([128, 4, 64], BF, tag="v")
                    nc.vector.scalar_tensor_tensor(
                        v[:], u[:, 0:4], 2.0, u[:, 4:8], OP.mult, OP.mult)
                    sdf = scp.tile([128, 4, 64], BF, tag="sdf")
                    nc.vector.tensor_tensor(sdf[:], v[:], u[:, 0:4],
                                            OP.subtract)
                    # gamma path (off critical chain)
                    gp_ = gps.tile([128, 4, 64], F32, tag="g")
                    for m in range(4):
                        for k in range(4):
                            nc.tensor.matmul(gp_[:, m], wki[:, k, m], h[:, k],
                                             start=(k == 0), stop=(k == 3))
                    gpre = scp.tile([128, 4, 64], BF, tag="gpre")
                    nc.vector.tensor_tensor(gpre[:], gp_[:], kib[:, :, col],
                                            OP.add)
                    gam = scp.tile([128, 4, 64], BF, tag="gam")
                    nc.scalar.activation(gam[:], gpre[:], AF.Sigmoid,
                                         scale=inv_s[2])
                    # p path
                    pp = pps.tile([128, 8, 64], F32, tag="p")
                    for m in range(8):
                        nc.tensor.matmul(pp[:, m], tco[:2, m, :], oh_co[:, col],
                                         start=True, stop=False)
                        for k in range(4):
                            nc.tensor.matmul(pp[:, m], wp12[:, k, m], sdf[:, k],
                                             start=False, stop=(k == 3))
                    w = scp.tile([128, 8, 64], BF, tag="w")
                    nc.scalar.activation(w[:], pp[:], AF.Sigmoid,
                                         scale=inv_s[1])
                    pv = scp.tile([128, 4, 64], BF, tag="pv")
                    nc.vector.scalar_tensor_tensor(
                        pv[:], w[:, 0:4], 2.0, w[:, 4:8], OP.mult, OP.mult)
                    pka = scp.tile([128, 4, 64], BF, tag="pka")
                    nc.vector.tensor_tensor(pka[:], pv[:], w[:, 0:4],
                                            OP.subtract)
                    # h update
                    t1 = scp.tile([128, 4, 64], BF, tag="t1")
                    nc.vector.tensor_tensor(t1[:], h[:], pka[:], OP.subtract)
                    t2 = scp.tile([128, 4, 64], BF, tag="t2")
                    nc.vector.tensor_tensor(t2[:], t1[:], gam[:], OP.mult)
                    h = scp.tile([128, 4, 64], BF, tag="h")
                    nc.vector.tensor_tensor(h[:], t2[:], pka[:], OP.add)
                    # y partial: x_{t+1} . h
                    if tt < TC - 1:
                        xn = xb[:, :, (tt + 1) * 64:(tt + 2) * 64]
                    else:
                        xn = chunks[c + 1][0][:, :, 0:64]
                    yv = scp.tile([128, 4, 64], F32, tag="yv")
                    nc.vector.tensor_tensor(yv[:], xn, h[:], OP.mult)
                    nc.vector.tensor_reduce(
                        prch[:, col], yv.rearrange("p m b -> p b m"),
                        mybir.AxisListType.X, OP.add)
                # chunk y output
                yp = yps.tile([1, TC * 64], F32, tag="yp")
                nc.tensor.matmul(yp[:], ones[:, 0:1], prch[:],
                                 start=True, stop=True)
                ych = scp.tile([1, TC * 64], F32, tag="ych")
                nc.scalar.activation(ych[:, 0:steps * 64],
                                     yp[:, 0:steps * 64], AF.Sigmoid)
                nc.sync.dma_start(
                    YT.ap().rearrange("t b -> (t b)")[None][
                        :, c * CHUNK:c * CHUNK + steps * 64],
                    ych[:, 0:steps * 64])
                nxt = c + 2
                if nxt < NCHUNK:
                    chunks[nxt] = produce(nxt)
            # final zero row y[:, S-1] = 0
            nc.sync.dma_start(
                YT.ap().rearrange("t b -> (t b)")[None][
                    :, (S - 1) * BC:S * BC], zrow[:])

    nc.compile()
    return nc


def prep_in_map(inputs, core, scales):
    ii = {k: np.asarray(v) for k, v in inputs.items()}
    sl = slice(core * BC, (core + 1) * BC)
    s_s, s_p, s_k = scales
    W_x, W_s1, W_s2 = ii['W_x'], ii['W_s1'], ii['W_s2']
    W_p1, W_p2, W_ki = ii['W_p1'], ii['W_p2'], ii['W_ki']

    q = ii['question_seq'][sl].astype(np.int64)
    cseq = ii['concept_seq'][sl].astype(np.int64)
    qd = ii['question_diff_seq'][sl].astype(np.int64)
    cd = ii['concept_diff_seq'][sl].astype(np.int64)
    co = ii['correct_seq'][sl].astype(np.int64)

    qf = q.T.ravel()      # t-major
    cf = cseq.T.ravel()
    qdf = qd.T.ravel().astype(np.float32).astype(bf16)
    cdf = cd.T.ravel().astype(np.float32).astype(bf16)
    cof = co.T.ravel().astype(np.float32).astype(bf16)

    ws12 = _wtile(np.concatenate([s_s * W_s1, 2 * s_s * W_s2], 1)).astype(wnp)
    wp12 = _wtile(np.concatenate([s_p * W_p1[:D], 2 * s_p * W_p2[:D]], 1)
                  ).astype(wnp)
    wki1 = _wtile(s_k * W_ki[:D]).astype(wnp)

    wqdr = _wtile(np.concatenate([W_x[2*D:3*D], s_k * W_ki[2*D:3*D]], 1)
                  ).reshape(128, 4, 1024).astype(bf16)
    bqd = np.concatenate([ii['b_x'], 0 * ii['b_ki']])[None].astype(bf16)
    wcdr = _wtile(np.concatenate([W_x[3*D:], s_k * W_ki[3*D:]], 1)
                  ).reshape(128, 4, 1024).astype(bf16)
    wcor = _wtile(np.concatenate([s_p * W_p1[D:], 2 * s_p * W_p2[D:]], 1)
                  ).reshape(128, 4, 1024).astype(bf16)
    bco = np.concatenate([s_p * ii['b_p1'], 2 * s_p * ii['b_p2']]
                         )[None].astype(bf16)
    wcok = _wtile(s_k * W_ki[D:2*D]).reshape(128, 4, 512).astype(bf16)
    bcok = (s_k * ii['b_ki'])[None].astype(bf16)
    tbs_row = np.concatenate([s_s * ii['b_s1'], 2 * s_s * ii['b_s2']])
    tbs = np.ascontiguousarray(
        np.broadcast_to(tbs_row.reshape(1, 8, 128), (2, 8, 128))).astype(bf16)

    h0 = ii['h0'][sl]  # [64, 512]
    h0t = np.ascontiguousarray(
        h0.T.reshape(4, 128, BC).transpose(1, 0, 2)).astype(bf16)

    return {
        'EQ': ii['E_q'].astype(bf16),
        'EC': ii['E_c'].astype(bf16),
        'QIDX': _wrap_idx(qf),
        'CIDX': _wrap_idx(cf),
        'QDI': np.ascontiguousarray(np.tile(qdf[None], (128, 1))),
        'CDI': np.ascontiguousarray(np.tile(cdf[None], (128, 1))),
        'COI': np.ascontiguousarray(np.tile(cof[None], (2, 1))),
        'WS12': ws12, 'WP12': wp12, 'WKI': wki1,
        'WXQ': _wtile(W_x[:D]).astype(bf16),
        'WXC': _wtile(W_x[D:2*D]).astype(bf16),
        'EQDT': _ttile(ii['E_qd']),
        'WQDR': wqdr, 'BQD': bqd,
        'ECDT': _ttile(ii['E_cd']),
        'WCDR': wcdr,
        'ECOT': _ttile(ii['E_corr']),
        'WCOR': wcor, 'BCO': bco,
        'WCOK': wcok, 'BCOK': bcok,
        'TBS': tbs,
        'SCL': np.array([[s_s, s_p, s_k, 0]], dtype=np.float32),
        'H0T': h0t,
    }


_nc_cache = {}


def _get_scales(inputs):
    ii = inputs
    s_s = _scale_for(np.concatenate(
        [np.asarray(ii['W_s1']), 2 * np.asarray(ii['W_s2'])], 1))
    s_p = _scale_for(np.concatenate(
        [np.asarray(ii['W_p1'][:D]), 2 * np.asarray(ii['W_p2'][:D])], 1))
    s_k = _scale_for(np.asarray(ii['W_ki'][:D]))
    return s_s, s_p, s_k


def run(inputs, trace=False):
    scales = _get_scales(inputs)
    key = scales
    if key not in _nc_cache:
        # stash inverse scales where build_program reads them
        class _NCFactory:
            pass
        bacc.Bacc._dimkt_inv_scales = [1.0 / scales[0], 1.0 / scales[1],
                                       1.0 / scales[2]]
        _nc_cache[key] = build_program()
    nc = _nc_cache[key]
    in_maps = [prep_in_map(inputs, c, scales) for c in range(NCORE)]
    last = None
    for attempt in range(3):
        try:
            res = bass_utils.run_bass_kernel_spmd(
                nc, in_maps, core_ids=list(range(NCORE)), trace=trace)
            break
        except Exception as e:
            last = e
    else:
        raise last
    yts = [res.results[c]["YT"] for c in range(NCORE)]   # each [200, 64]
    y = np.concatenate([yt.T for yt in yts], axis=0)     # [512, 200]
    return y.astype(np.float32), res


def kernel(**inputs):
    y, _ = run(inputs)
    return y
